# revision 8
# baseline (speedup 1.0000x reference)
"""Bass/Tile SPMD kernel for nn_CEN_BRL_22763326668900 on 8 trn2 NeuronCores.

Reference computation (see problem):
  phi = relu(ctx @ w1.T + b1) @ w2.T + b2            [4096, 256]
  pre = S.T @ Wa.T                                   [2048, 256]
  16 LSTM steps with x_t rank-1:
      x_0 = ones, x_{t+1} = S[:, idx_t] broadcast over columns
  =>  x @ w_ih.T == outer(s_col, ws), ws = w_ih.sum(axis=1)   (host-precomputed)
  Per step: z = h @ w_hh.T + outer + bias; gates -> c,h; e = mean_rows(h);
  hid = relu(pre + e @ Wb.T + b1a); scores = hid @ w2a.T; logp = log_softmax;
  idx = argmax.

Sharding: data-parallel over 4096 train rows (512/core). Per step one tiny
AllGather of per-core av partials (av = rowsum(h) @ Wb.T/4096). Attention MLP,
softmax and argmax replicated on every core; each core gathers its own
S[:, idx] shard column via indirect DMA from S_shard.T.

Matmuls run in fp16 (full PE rate; 4-byte matmuls hit a walrus codegen bug
with sync waits). Top-2 score margin is ~0.09 vs ~1e-3 worst-case fp16 error,
so the argmax chain is stable; accumulation stays fp32 in PSUM.
"""

import os
import sys
import numpy as np

sys.path.insert(0, "/opt/trn_rl_repo")

NCORES = 8
N_TRAIN, N_FEAT, N_HID, ENC = 4096, 64, 256, 256
N_ANTES, ATT_H, MAX_LEN = 2048, 256, 16
R = N_TRAIN // NCORES  # 512 rows per core
G4 = 4 * ENC  # 1024


def build_nc():
    import concourse.bass as bass
    import concourse.bacc as bacc
    import concourse.tile as tile
    from concourse import mybir
    from contextlib import ExitStack

    f32 = mybir.dt.float32
    f16 = mybir.dt.float16
    u32 = mybir.dt.uint32
    AF = mybir.ActivationFunctionType
    ALU = mybir.AluOpType
    AX = mybir.AxisListType

    nc = bacc.Bacc(num_devices=NCORES)

    # ---- kernel I/O ----
    ctxT_d = nc.dram_tensor("ctxT", [N_FEAT, R], f16, kind="ExternalInput")
    S_kxn_d = nc.dram_tensor("S_kxn", [128, 4, N_ANTES], f16, kind="ExternalInput")
    S_T_d = nc.dram_tensor("S_T", [N_ANTES, R], f16, kind="ExternalInput")
    WaT_d = nc.dram_tensor("WaT", [128, 4, ATT_H], f16, kind="ExternalInput")
    ew1T_d = nc.dram_tensor("enc_w1T", [N_FEAT, N_HID], f16, kind="ExternalInput")
    eb1_d = nc.dram_tensor("enc_b1p", [128, 2], f32, kind="ExternalInput")
    ew2T_d = nc.dram_tensor("enc_w2T", [128, 2, ENC], f16, kind="ExternalInput")
    eb2_d = nc.dram_tensor("enc_b2p", [128, 2], f32, kind="ExternalInput")
    whhT_d = nc.dram_tensor("w_hhT", [128, 2, G4], f16, kind="ExternalInput")
    ws_d = nc.dram_tensor("ws", [1, G4], f16, kind="ExternalInput")
    biasp_d = nc.dram_tensor("biasp", [128, 8], f32, kind="ExternalInput")
    WbT_d = nc.dram_tensor("WbT", [128, 2, ATT_H], f16, kind="ExternalInput")
    ab1_d = nc.dram_tensor("ab1p", [128, 2], f32, kind="ExternalInput")
    w2p_d = nc.dram_tensor("w2p", [128, 2], f16, kind="ExternalInput")
    id8_d = nc.dram_tensor("ident8", [8, 8], f16, kind="ExternalInput")
    out_d = nc.dram_tensor("out", [MAX_LEN, N_ANTES], f32, kind="ExternalOutput")

    # internal DRAM for collectives
    cc_av_in = nc.dram_tensor("cc_av_in", [1, ATT_H], f16)
    cc_av_out = nc.dram_tensor("cc_av_out", [NCORES, ATT_H], f16, addr_space="Shared")
    cc_pre_in = nc.dram_tensor("cc_pre_in", [ATT_H, N_ANTES], f32)
    cc_pre_out = nc.dram_tensor(
        "cc_pre_out", [ATT_H, N_ANTES], f32, addr_space="Shared"
    )
    groups = [list(range(NCORES))]

    with tile.TileContext(nc, num_cores=NCORES) as tc, ExitStack() as ctx:
        const = ctx.enter_context(tc.tile_pool(name="const", bufs=1))
        state = ctx.enter_context(tc.tile_pool(name="state", bufs=1))
        work = ctx.enter_context(tc.tile_pool(name="work", bufs=2))
        psum = ctx.enter_context(tc.tile_pool(name="psum", bufs=8, space="PSUM"))

        # ---- load constants ----
        def load(shape, dram, tag, dt=f32):
            t = const.tile(shape, dt, tag=tag)
            nc.sync.dma_start(out=t[:], in_=dram[:])
            return t

        ctxT = load([N_FEAT, R], ctxT_d, "ctxT", f16)
        ew1T = load([N_FEAT, N_HID], ew1T_d, "ew1T", f16)
        eb1 = load([128, 2], eb1_d, "eb1")
        ew2T = load([128, 2, ENC], ew2T_d, "ew2T", f16)
        eb2 = load([128, 2], eb2_d, "eb2")
        whhT = load([128, 2, G4], whhT_d, "whhT", f16)
        ws = load([1, G4], ws_d, "ws", f16)
        biasp = load([128, 8], biasp_d, "biasp")
        WbT = load([128, 2, ATT_H], WbT_d, "WbT", f16)
        ab1 = load([128, 2], ab1_d, "ab1")
        w2p = load([128, 2], w2p_d, "w2p", f16)
        id8 = load([8, 8], id8_d, "id8", f16)

        # ---- persistent state ----
        hT = state.tile([128, 2, R], f16)  # h transposed: [enc(128x2), rows]
        cT = state.tile([128, 2, R], f32)
        s_col2 = state.tile([2, R], f16)  # row 0 = S_shard[:, idx]
        pre0 = state.tile([128, N_ANTES], f32)  # preT rows 0:128
        pre1 = state.tile([128, N_ANTES], f32)  # preT rows 128:256
        shist = state.tile([MAX_LEN, N_ANTES], f32)  # raw scores per step
        hsum = state.tile([128, 2, 1], f32)
        hsum16 = state.tile([128, 2, 1], f16)
        avb = state.tile([128, 2, 1], f32)
        off2 = state.tile([2, 1], u32)
        mx8 = state.tile([1, 8], f32)
        mi8 = state.tile([1, 8], u32)

        nc.vector.memset(cT[:], 0.0)
        nc.vector.memset(s_col2[:], 1.0)  # x_0 = ones
        nc.vector.memset(off2[:], 0)

        # ---- phi = encoder(context) -> h_0 (transposed layout) ----
        a1_tiles = []
        for m in range(2):
            p = psum.tile([128, R], f32, tag="bank")
            nc.tensor.matmul(
                p[:], ew1T[:, m * 128 : (m + 1) * 128], ctxT[:],
                start=True, stop=True,
            )
            a1 = work.tile([128, R], f16, tag="a1")
            nc.scalar.activation(a1[:], p[:], AF.Relu, bias=eb1[:, m : m + 1])
            a1_tiles.append(a1)
        for m in range(2):
            p = psum.tile([128, R], f32, tag="bank")
            for k in range(2):
                nc.tensor.matmul(
                    p[:], ew2T[:, k, m * 128 : (m + 1) * 128], a1_tiles[k][:],
                    start=(k == 0), stop=(k == 1),
                )
            nc.scalar.activation(
                hT[:, m, :], p[:], AF.Identity, bias=eb2[:, m : m + 1]
            )

        # ---- pre partial = Wa_shard @ S_shard, then AllReduce ----
        with tc.tile_pool(name="spool", bufs=1) as spool:
            S_sb = spool.tile([128, 4, N_ANTES], f16)
            nc.sync.dma_start(out=S_sb[:], in_=S_kxn_d[:])
            WaT_sb = spool.tile([128, 4, ATT_H], f16)
            nc.sync.dma_start(out=WaT_sb[:], in_=WaT_d[:])
            for m in range(2):
                prep = work.tile([128, N_ANTES], f32, tag="hid%d" % m)
                for n in range(4):
                    p = psum.tile([128, 512], f32, tag="bank")
                    for k in range(4):
                        nc.tensor.matmul(
                            p[:],
                            WaT_sb[:, k, m * 128 : (m + 1) * 128],
                            S_sb[:, k, n * 512 : (n + 1) * 512],
                            start=(k == 0), stop=(k == 3),
                        )
                    if n % 2 == 0:
                        nc.scalar.copy(prep[:, n * 512 : (n + 1) * 512], p[:])
                    else:
                        nc.vector.tensor_copy(prep[:, n * 512 : (n + 1) * 512], p[:])
                nc.sync.dma_start(
                    out=cc_pre_in[m * 128 : (m + 1) * 128, :], in_=prep[:]
                )
        nc.gpsimd.collective_compute(
            "AllReduce", ALU.add, replica_groups=groups,
            ins=[cc_pre_in[:]], outs=[cc_pre_out[:]],
        )
        nc.sync.dma_start(out=pre0[:], in_=cc_pre_out[0:128, :])
        nc.sync.dma_start(out=pre1[:], in_=cc_pre_out[128:256, :])

        # ---- 16 LSTM + attention steps ----
        # m-tile order: e=0 gates first (i0,f0,g0,o0), then e=1
        m_order = [0, 2, 4, 6, 1, 3, 5, 7]
        for t in range(MAX_LEN):
            gate = {}
            for m in m_order:
                p = psum.tile([128, R], f32, tag="bank")
                nc.tensor.matmul(
                    p[:], whhT[:, 0, m * 128 : (m + 1) * 128], hT[:, 0, :],
                    start=True, stop=False,
                )
                nc.tensor.matmul(
                    p[:], whhT[:, 1, m * 128 : (m + 1) * 128], hT[:, 1, :],
                    start=False, stop=False,
                )
                nc.tensor.matmul(
                    p[:], ws[0:1, m * 128 : (m + 1) * 128], s_col2[0:1, :],
                    start=False, stop=True,
                )
                g, e = divmod(m, 2)
                func = AF.Tanh if g == 2 else AF.Sigmoid
                gt = work.tile([128, R], f32, tag="g%d%d" % (g, e))
                nc.scalar.activation(gt[:], p[:], func, bias=biasp[:, m : m + 1])
                gate[(g, e)] = gt
                if m in (6, 7):  # all four gates of chunk e ready
                    ig = work.tile([128, R], f32, tag="ig%d" % e)
                    nc.vector.tensor_mul(ig[:], gate[(0, e)][:], gate[(2, e)][:])
                    cf = work.tile([128, R], f32, tag="cf%d" % e)
                    nc.vector.tensor_mul(cf[:], cT[:, e, :], gate[(1, e)][:])
                    nc.vector.tensor_add(cT[:, e, :], ig[:], cf[:])
                    th = work.tile([128, R], f32, tag="th%d" % e)
                    nc.scalar.activation(th[:], cT[:, e, :], AF.Tanh)
                    nc.vector.tensor_mul(hT[:, e, :], gate[(3, e)][:], th[:])
                    nc.vector.reduce_sum(hsum[:, e, :], hT[:, e, :], axis=AX.X)

            # av partial = rowsum(h) @ (Wb.T/4096); AllGather partials
            nc.vector.tensor_copy(hsum16[:], hsum[:])
            pav = psum.tile([1, ATT_H], f32, tag="bank")
            nc.tensor.matmul(
                pav[:], hsum16[:, 0, :], WbT[:, 0, :], start=True, stop=False
            )
            nc.tensor.matmul(
                pav[:], hsum16[:, 1, :], WbT[:, 1, :], start=False, stop=True
            )
            avp_sb = work.tile([1, ATT_H], f16, tag="avp")
            nc.vector.tensor_copy(avp_sb[:], pav[:])
            nc.sync.dma_start(out=cc_av_in[:], in_=avp_sb[:])
            nc.gpsimd.collective_compute(
                "AllGather", ALU.bypass, replica_groups=groups,
                ins=[cc_av_in[:]], outs=[cc_av_out[:]],
            )
            av_all = work.tile([NCORES, ATT_H], f16, tag="avall")
            nc.sync.dma_start(out=av_all[:], in_=cc_av_out[:])
            for e in range(2):
                pt = psum.tile([128, 8], f16, tag="bank")
                nc.tensor.transpose(
                    pt[:], av_all[0:NCORES, e * 128 : (e + 1) * 128], id8[:]
                )
                avs = work.tile([128, 1], f32, tag="avs%d" % e)
                nc.vector.reduce_sum(avs[:], pt[:], axis=AX.X)
                nc.vector.tensor_add(avb[:, e, :], avs[:], ab1[:, e : e + 1])

            # hid = relu(pre + avb) ; scores = w2 . hid
            hid0 = work.tile([128, N_ANTES], f16, tag="hid0")
            nc.scalar.activation(hid0[:], pre0[:], AF.Relu, bias=avb[:, 0, :])
            hid1 = work.tile([128, N_ANTES], f16, tag="hid1")
            nc.vector.tensor_scalar(
                hid1[:], pre1[:], avb[:, 1, :], 0.0, op0=ALU.add, op1=ALU.max
            )
            scs = work.tile([1, N_ANTES], f32, tag="scs")
            for n in range(4):
                ps = psum.tile([1, 512], f32, tag="bank")
                nc.tensor.matmul(
                    ps[:], w2p[:, 0:1], hid0[:, n * 512 : (n + 1) * 512],
                    start=True, stop=False,
                )
                nc.tensor.matmul(
                    ps[:], w2p[:, 1:2], hid1[:, n * 512 : (n + 1) * 512],
                    start=False, stop=True,
                )
                dst = scs[0:1, n * 512 : (n + 1) * 512]
                if n % 2 == 0:
                    nc.scalar.copy(dst, ps[:])
                else:
                    nc.vector.tensor_copy(dst, ps[:])
            nc.sync.dma_start(out=shist[t : t + 1, :], in_=scs[:])

            if t < MAX_LEN - 1:
                # argmax over 2048 scores, then gather S_shard[:, idx]
                nc.vector.max(mx8[:], scs[:])
                nc.vector.max_index(mi8[:], mx8[:], scs[:])
                nc.vector.tensor_copy(off2[0:1, 0:1], mi8[0:1, 0:1])
                nc.gpsimd.indirect_dma_start(
                    out=s_col2[:],
                    out_offset=None,
                    in_=S_T_d[:],
                    in_offset=bass.IndirectOffsetOnAxis(ap=off2[:, 0:1], axis=0),
                )

        # ---- batched log_softmax over all 16 steps ----
        mx = state.tile([MAX_LEN, 1], f32)
        nc.vector.reduce_max(mx[:], shist[:], axis=AX.X)
        negm = state.tile([MAX_LEN, 1], f32)
        nc.vector.tensor_scalar_mul(negm[:], mx[:], -1.0)
        etile = work.tile([MAX_LEN, N_ANTES], f32, tag="etile")
        sume = state.tile([MAX_LEN, 1], f32)
        nc.scalar.activation(
            etile[:], shist[:], AF.Exp, bias=negm[:], accum_out=sume[:]
        )
        lsum = state.tile([MAX_LEN, 1], f32)
        nc.scalar.activation(lsum[:], sume[:], AF.Ln)
        offc = state.tile([MAX_LEN, 1], f32)
        nc.vector.tensor_add(offc[:], mx[:], lsum[:])
        nc.vector.tensor_scalar_mul(offc[:], offc[:], -1.0)
        logp = work.tile([MAX_LEN, N_ANTES], f32, tag="logp")
        nc.scalar.activation(logp[:], shist[:], AF.Identity, bias=offc[:])
        nc.sync.dma_start(out=out_d[:], in_=logp[:])

    nc.compile()
    return nc


def pack2(a):
    """[nk*128, X] -> [128, nk, X] so that a[k*128+p, x] == out[p, k, x]."""
    rows, X = a.shape
    nk = rows // 128
    return np.ascontiguousarray(a.reshape(nk, 128, X).transpose(1, 0, 2))


def colpack(v):
    """[nk*128] -> [128, nk] per-partition bias layout."""
    n = v.shape[0]
    nk = n // 128
    return np.ascontiguousarray(v.reshape(nk, 128).T)


def make_in_maps(inputs):
    f32 = np.float32
    f16 = np.float16
    context = np.ascontiguousarray(np.asarray(inputs["context"], f32))
    S = np.ascontiguousarray(np.asarray(inputs["S"], f32))
    enc_w1 = np.asarray(inputs["enc_w1"], f32)
    enc_b1 = np.asarray(inputs["enc_b1"], f32)
    enc_w2 = np.asarray(inputs["enc_w2"], f32)
    enc_b2 = np.asarray(inputs["enc_b2"], f32)
    w_ih = np.asarray(inputs["w_ih"], f32)
    w_hh = np.asarray(inputs["w_hh"], f32)
    b_ih = np.asarray(inputs["b_ih"], f32)
    b_hh = np.asarray(inputs["b_hh"], f32)
    att_w1 = np.asarray(inputs["att_w1"], f32)
    att_b1 = np.asarray(inputs["att_b1"], f32)
    att_w2 = np.asarray(inputs["att_w2"], f32)

    shared = {
        "enc_w1T": np.ascontiguousarray(enc_w1.T).astype(f16),
        "enc_b1p": colpack(enc_b1),
        "enc_w2T": pack2(np.ascontiguousarray(enc_w2.T)).astype(f16),
        "enc_b2p": colpack(enc_b2),
        "w_hhT": pack2(np.ascontiguousarray(w_hh.T)).astype(f16),
        "ws": w_ih.sum(axis=1, dtype=np.float64).astype(f32)[None, :].astype(f16),
        "biasp": colpack(b_ih + b_hh),
        "WbT": pack2(np.ascontiguousarray(att_w1[:, N_TRAIN:].T) / N_TRAIN).astype(
            f16
        ),
        "ab1p": colpack(att_b1),
        "w2p": colpack(att_w2[0]).astype(f16),
        "ident8": np.eye(8, dtype=f16),
    }
    in_maps = []
    for c in range(NCORES):
        rows = slice(c * R, (c + 1) * R)
        m = dict(shared)
        m["ctxT"] = np.ascontiguousarray(context[rows].T).astype(f16)
        m["S_kxn"] = pack2(S[rows]).astype(f16)
        m["S_T"] = np.ascontiguousarray(S[rows].T).astype(f16)
        m["WaT"] = pack2(np.ascontiguousarray(att_w1[:, :N_TRAIN][:, rows].T)).astype(
            f16
        )
        in_maps.append(m)
    return in_maps


_NC = None


def kernel(**inputs):
    global _NC
    from concourse.bass_utils import run_bass_kernel_spmd

    if _NC is None:
        _NC = build_nc()
    in_maps = make_in_maps(inputs)
    res = run_bass_kernel_spmd(
        _NC, in_maps, list(range(NCORES)),
        trace=bool(int(os.environ.get("KERNEL_TRACE", "0"))),
    )
    out = np.asarray(res.results[0]["out"], np.float32)
    if res.exec_time_ns is not None:
        print(f"HW exec time: {res.exec_time_ns} ns")
    return out


# revision 11
# speedup vs baseline: 1.0820x; 1.0820x over previous
"""Bass/Tile SPMD kernel for nn_CEN_BRL_22763326668900 on 8 trn2 NeuronCores.

Reference computation (see problem):
  phi = relu(ctx @ w1.T + b1) @ w2.T + b2            [4096, 256]
  pre = S.T @ Wa.T                                   [2048, 256]
  16 LSTM steps with x_t rank-1:
      x_0 = ones, x_{t+1} = S[:, idx_t] broadcast over columns
  =>  x @ w_ih.T == outer(s_col, ws), ws = w_ih.sum(axis=1)   (host-precomputed)
  Per step: z = h @ w_hh.T + outer + bias; gates -> c,h; e = mean_rows(h);
  hid = relu(pre + e @ Wb.T + b1a); scores = hid @ w2a.T; logp = log_softmax;
  idx = argmax.

Sharding: data-parallel over 4096 train rows (512/core). Per step one tiny
AllGather of per-core av partials (av = rowsum(h) @ Wb.T/4096). Attention MLP,
softmax and argmax replicated on every core; each core gathers its own
S[:, idx] shard column via indirect DMA from (S_shard.T with a ones row
appended; the second gather offset pins row 1 of the rhs to ones so a single
K=2 matmul applies both the outer product and the gate bias).

Matmuls run in fp16 (full PE rate; 4-byte matmuls hit a walrus codegen bug
with sync waits). Top-2 score margin is ~0.09 vs ~1e-3 worst-case fp16 error,
so the argmax chain is stable; accumulation stays fp32 in PSUM.

Pipelining: the next step's w_hh matmuls for gates i,f,g are emitted right
after the AllGather is issued, so the PE fills the collective latency window;
gate o's matmuls take the attention psum slot after the score matmuls drain.
"""

import os
import sys
import numpy as np

sys.path.insert(0, "/opt/trn_rl_repo")

NCORES = 8
N_TRAIN, N_FEAT, N_HID, ENC = 4096, 64, 256, 256
N_ANTES, ATT_H, MAX_LEN = 2048, 256, 16
R = N_TRAIN // NCORES  # 512 rows per core
G4 = 4 * ENC  # 1024


def build_nc():
    import concourse.bass as bass
    import concourse.bacc as bacc
    import concourse.tile as tile
    from concourse import mybir
    from contextlib import ExitStack

    f32 = mybir.dt.float32
    f16 = mybir.dt.float16
    u32 = mybir.dt.uint32
    AF = mybir.ActivationFunctionType
    ALU = mybir.AluOpType
    AX = mybir.AxisListType

    nc = bacc.Bacc(num_devices=NCORES)

    # ---- kernel I/O ----
    ctxT_d = nc.dram_tensor("ctxT", [N_FEAT, R], f16, kind="ExternalInput")
    S_kxn_d = nc.dram_tensor("S_kxn", [128, 4, N_ANTES], f16, kind="ExternalInput")
    # S_shard.T plus a trailing all-ones row (gather offset 2048 -> ones)
    S_T_d = nc.dram_tensor("S_T", [N_ANTES + 1, R], f16, kind="ExternalInput")
    WaT_d = nc.dram_tensor("WaT", [128, 4, ATT_H], f16, kind="ExternalInput")
    ew1T_d = nc.dram_tensor("enc_w1T", [N_FEAT, N_HID], f16, kind="ExternalInput")
    eb1_d = nc.dram_tensor("enc_b1p", [128, 2], f32, kind="ExternalInput")
    ew2T_d = nc.dram_tensor("enc_w2T", [128, 2, ENC], f16, kind="ExternalInput")
    eb2_d = nc.dram_tensor("enc_b2p", [128, 2], f32, kind="ExternalInput")
    whhT_d = nc.dram_tensor("w_hhT", [128, 2, G4], f16, kind="ExternalInput")
    # row 0 = w_ih.sum(1), row 1 = b_ih + b_hh  (K=2 outer+bias matmul lhsT)
    wsb_d = nc.dram_tensor("wsb", [2, G4], f16, kind="ExternalInput")
    WbT_d = nc.dram_tensor("WbT", [128, 2, ATT_H], f16, kind="ExternalInput")
    ab1_d = nc.dram_tensor("ab1p", [128, 2], f32, kind="ExternalInput")
    w2p_d = nc.dram_tensor("w2p", [128, 2], f16, kind="ExternalInput")
    id8_d = nc.dram_tensor("ident8", [8, 8], f16, kind="ExternalInput")
    # gather offsets init: [0, 2048] (row 1 always reads the ones row)
    offi_d = nc.dram_tensor("off_init", [2, 1], u32, kind="ExternalInput")
    iota4_d = nc.dram_tensor("iota4", [1, 4], f32, kind="ExternalInput")
    out_d = nc.dram_tensor("out", [MAX_LEN, N_ANTES], f32, kind="ExternalOutput")

    # internal DRAM for collectives
    cc_av_in = nc.dram_tensor("cc_av_in", [1, ATT_H], f16)
    cc_av_out = nc.dram_tensor("cc_av_out", [NCORES, ATT_H], f16, addr_space="Shared")
    cc_pre_in = nc.dram_tensor("cc_pre_in", [ATT_H, N_ANTES], f16)
    cc_pre_out = nc.dram_tensor(
        "cc_pre_out", [ATT_H, N_ANTES], f16, addr_space="Shared"
    )
    groups = [list(range(NCORES))]

    with tile.TileContext(nc, num_cores=NCORES) as tc, ExitStack() as ctx:
        const = ctx.enter_context(tc.tile_pool(name="const", bufs=1))
        state = ctx.enter_context(tc.tile_pool(name="state", bufs=1))
        work = ctx.enter_context(tc.tile_pool(name="work", bufs=2))
        # 3 two-bank slots for LSTM gate psums (i,f,g prefill; o borrows apool)
        zpool = ctx.enter_context(tc.tile_pool(name="zpool", bufs=3, space="PSUM"))
        apool = ctx.enter_context(tc.tile_pool(name="apool", bufs=2, space="PSUM"))

        # ---- load constants ----
        def load(shape, dram, tag, dt=f32):
            t = const.tile(shape, dt, tag=tag)
            nc.sync.dma_start(out=t[:], in_=dram[:])
            return t

        ctxT = load([N_FEAT, R], ctxT_d, "ctxT", f16)
        ew1T = load([N_FEAT, N_HID], ew1T_d, "ew1T", f16)
        eb1 = load([128, 2], eb1_d, "eb1")
        ew2T = load([128, 2, ENC], ew2T_d, "ew2T", f16)
        eb2 = load([128, 2], eb2_d, "eb2")
        whhT = load([128, 2, G4], whhT_d, "whhT", f16)
        wsb = load([2, G4], wsb_d, "wsb", f16)
        WbT = load([128, 2, ATT_H], WbT_d, "WbT", f16)
        ab1 = load([128, 2], ab1_d, "ab1")
        w2p = load([128, 2], w2p_d, "w2p", f16)
        id8 = load([8, 8], id8_d, "id8", f16)
        iota4 = load([1, 4], iota4_d, "iota4")

        # ---- persistent state ----
        hT = state.tile([128, 2, R], f16)  # h transposed: [enc(128x2), rows]
        cT = state.tile([128, 2, R], f32)
        s_col2 = state.tile([2, R], f16)  # row0 = S_shard[:, idx], row1 = ones
        pre0 = state.tile([128, N_ANTES], f16)  # preT rows 0:128
        pre1 = state.tile([128, N_ANTES], f16)  # preT rows 128:256
        shist = state.tile([MAX_LEN, N_ANTES], f32)  # raw scores per step
        hsum = state.tile([128, 2], f32)
        hsum16 = state.tile([128, 2], f16)
        avb = state.tile([128, 2, 1], f32)
        off2 = state.tile([2, 1], u32)
        cmax = state.tile([1, 4, 8], f32)  # per-512-chunk top-8 score values
        cmif = state.tile([1, 4], f32)  # per-chunk argmax as f32
        gmx = state.tile([1, 1], f32)
        eqm = state.tile([1, 4], f32)
        cix = state.tile([1, 1], f32)
        msk = state.tile([1, 4], f32)
        jsel = state.tile([1, 1], f32)
        idxf = state.tile([1, 1], f32)

        nc.vector.memset(cT[:], 0.0)
        nc.vector.memset(s_col2[:], 1.0)  # x_0 = ones (and row1 stays ones)
        nc.sync.dma_start(out=off2[:], in_=offi_d[:])

        # ---- phi = encoder(context) -> h_0 (transposed layout) ----
        a1_tiles = []
        for m in range(2):
            p = apool.tile([128, R], f32, tag="abank")
            nc.tensor.matmul(
                p[:], ew1T[:, m * 128 : (m + 1) * 128], ctxT[:],
                start=True, stop=True,
            )
            a1 = work.tile([128, R], f16, tag="a1")
            nc.scalar.activation(a1[:], p[:], AF.Relu, bias=eb1[:, m : m + 1])
            a1_tiles.append(a1)
        for m in range(2):
            p = apool.tile([128, R], f32, tag="abank")
            for k in range(2):
                nc.tensor.matmul(
                    p[:], ew2T[:, k, m * 128 : (m + 1) * 128], a1_tiles[k][:],
                    start=(k == 0), stop=(k == 1),
                )
            nc.scalar.activation(
                hT[:, m, :], p[:], AF.Identity, bias=eb2[:, m : m + 1]
            )

        # ---- pre partial = Wa_shard @ S_shard, then AllReduce (fp16) ----
        with tc.tile_pool(name="spool", bufs=1) as spool:
            S_sb = spool.tile([128, 4, N_ANTES], f16)
            nc.sync.dma_start(out=S_sb[:], in_=S_kxn_d[:])
            WaT_sb = spool.tile([128, 4, ATT_H], f16)
            nc.sync.dma_start(out=WaT_sb[:], in_=WaT_d[:])
            for m in range(2):
                prep = work.tile([128, N_ANTES], f16, tag="hid%d" % m)
                for n in range(4):
                    p = apool.tile([128, 512], f32, tag="abank")
                    for k in range(4):
                        nc.tensor.matmul(
                            p[:],
                            WaT_sb[:, k, m * 128 : (m + 1) * 128],
                            S_sb[:, k, n * 512 : (n + 1) * 512],
                            start=(k == 0), stop=(k == 3),
                        )
                    if n % 2 == 0:
                        nc.scalar.copy(prep[:, n * 512 : (n + 1) * 512], p[:])
                    else:
                        nc.vector.tensor_copy(prep[:, n * 512 : (n + 1) * 512], p[:])
                nc.sync.dma_start(
                    out=cc_pre_in[m * 128 : (m + 1) * 128, :], in_=prep[:]
                )
        nc.gpsimd.collective_compute(
            "AllReduce", ALU.add, replica_groups=groups,
            ins=[cc_pre_in[:]], outs=[cc_pre_out[:]],
        )
        nc.sync.dma_start(out=pre0[:], in_=cc_pre_out[0:128, :])
        nc.sync.dma_start(out=pre1[:], in_=cc_pre_out[128:256, :])

        # ---- 16 LSTM + attention steps (software pipelined on PE) ----
        # zp[g] is gate g's [128, 2, 512] psum: [p, e, i] = z[gate g, 128e+p, i]
        zp = [None] * 4

        def z_kmm(g, pool, tag):
            p = pool.tile([128, 2, 512], f32, tag=tag)
            for e in range(2):
                m = 2 * g + e
                for k in range(2):
                    nc.tensor.matmul(
                        p[:, e, :],
                        whhT[:, k, m * 128 : (m + 1) * 128],
                        hT[:, k, :],
                        start=(k == 0), stop=False,
                    )
            return p

        def z_outer(g):
            for e in range(2):
                m = 2 * g + e
                nc.tensor.matmul(
                    zp[g][:, e, :],
                    wsb[0:2, m * 128 : (m + 1) * 128],
                    s_col2[0:2, :],
                    start=False, stop=True,
                )

        # step 0 prologue: all four gate psums (s_col = ones already valid).
        # Gate o's k-matmuls wait for the zpool slot freed once gate i's
        # activation has consumed its psum, so close i,f,g's groups first.
        for g in range(3):
            zp[g] = z_kmm(g, zpool, "zbank")
        for g in range(3):
            z_outer(g)
        zp[3] = z_kmm(3, zpool, "zbank")
        z_outer(3)

        for t in range(MAX_LEN):
            # ---- gates -> c, h, rowsum(h) ----
            sig = [None] * 4
            for g in range(4):
                func = AF.Tanh if g == 2 else AF.Sigmoid
                gt = work.tile([128, 2, 512], f32, tag="g%d" % g)
                nc.scalar.activation(gt[:], zp[g][:], func)
                sig[g] = gt
            ig = work.tile([128, 2, 512], f32, tag="ig")
            nc.vector.tensor_mul(ig[:], sig[0][:], sig[2][:])
            cf = work.tile([128, 2, 512], f32, tag="cf")
            nc.vector.tensor_mul(cf[:], cT[:], sig[1][:])
            nc.vector.tensor_add(cT[:], ig[:], cf[:])
            th = work.tile([128, 2, 512], f32, tag="th")
            nc.scalar.activation(th[:], cT[:], AF.Tanh)
            nc.vector.tensor_mul(hT[:], sig[3][:], th[:])
            nc.vector.reduce_sum(hsum[:], hT[:], axis=AX.X)
            nc.vector.tensor_copy(hsum16[:], hsum[:])

            # ---- av partial + AllGather ----
            pav = apool.tile([1, ATT_H], f32, tag="abank")
            nc.tensor.matmul(
                pav[:], hsum16[:, 0:1], WbT[:, 0, :], start=True, stop=False
            )
            nc.tensor.matmul(
                pav[:], hsum16[:, 1:2], WbT[:, 1, :], start=False, stop=True
            )
            avp_sb = work.tile([1, ATT_H], f16, tag="avp")
            nc.vector.tensor_copy(avp_sb[:], pav[:])
            nc.sync.dma_start(out=cc_av_in[:], in_=avp_sb[:])
            nc.gpsimd.collective_compute(
                "AllGather", ALU.bypass, replica_groups=groups,
                ins=[cc_av_in[:]], outs=[cc_av_out[:]],
            )

            # prefill next step's w_hh matmuls for gates i,f,g under the AG
            if t < MAX_LEN - 1:
                for g in range(3):
                    zp[g] = z_kmm(g, zpool, "zbank")

            av_all = work.tile([NCORES, ATT_H], f16, tag="avall")
            nc.sync.dma_start(out=av_all[:], in_=cc_av_out[:])
            for e in range(2):
                pt = apool.tile([128, 8], f16, tag="abank")
                nc.tensor.transpose(
                    pt[:], av_all[0:NCORES, e * 128 : (e + 1) * 128], id8[:]
                )
                avs = work.tile([128, 1], f32, tag="avs%d" % e)
                nc.vector.reduce_sum(avs[:], pt[:], axis=AX.X)
                nc.vector.tensor_add(avb[:, e, :], avs[:], ab1[:, e : e + 1])

            # ---- hid = relu(pre + avb) ; scores = w2 . hid ----
            hid0 = work.tile([128, N_ANTES], f16, tag="hid0")
            nc.scalar.activation(hid0[:], pre0[:], AF.Relu, bias=avb[:, 0, :])
            hid1 = work.tile([128, N_ANTES], f16, tag="hid1")
            nc.vector.tensor_scalar(
                hid1[:], pre1[:], avb[:, 1, :], 0.0, op0=ALU.add, op1=ALU.max
            )
            scs = work.tile([1, N_ANTES], f32, tag="scs")
            last = t == MAX_LEN - 1
            for n in range(4):
                ps = apool.tile([1, 512], f32, tag="abank")
                nc.tensor.matmul(
                    ps[:], w2p[:, 0:1], hid0[:, n * 512 : (n + 1) * 512],
                    start=True, stop=False,
                )
                nc.tensor.matmul(
                    ps[:], w2p[:, 1:2], hid1[:, n * 512 : (n + 1) * 512],
                    start=False, stop=True,
                )
                dst = scs[0:1, n * 512 : (n + 1) * 512]
                if n % 2 == 0:
                    nc.scalar.copy(dst, ps[:])
                else:
                    nc.vector.tensor_copy(dst, ps[:])
                if not last:
                    # per-chunk top-8 + argmax, pipelined with later chunks
                    nc.vector.max(cmax[:, n, :], dst)
                    mi = work.tile([1, 8], u32, tag="mi%d" % n)
                    nc.vector.max_index(mi[:], cmax[:, n, :], dst)
                    nc.vector.tensor_copy(cmif[0:1, n : n + 1], mi[0:1, 0:1])
            nc.sync.dma_start(out=shist[t : t + 1, :], in_=scs[:])

            if not last:
                # combine the 4 chunk argmaxes (single-lane, short rows)
                nc.vector.reduce_max(gmx[:], cmax[0:1, :, 0], axis=AX.X)
                nc.vector.tensor_scalar(
                    eqm[:], cmax[0:1, :, 0], gmx[0:1, 0:1], None,
                    op0=ALU.is_equal,
                )
                nc.vector.tensor_mul(msk[:], eqm[:], iota4[:])
                nc.vector.reduce_max(cix[:], msk[:], axis=AX.X)
                nc.vector.tensor_mul(msk[:], eqm[:], cmif[:])
                nc.vector.reduce_max(jsel[:], msk[:], axis=AX.X)
                nc.vector.tensor_scalar(
                    idxf[:], cix[:], 512.0, None, op0=ALU.mult
                )
                nc.vector.tensor_add(idxf[:], idxf[:], jsel[:])
                nc.vector.tensor_copy(off2[0:1, 0:1], idxf[:])
                nc.gpsimd.indirect_dma_start(
                    out=s_col2[:],
                    out_offset=None,
                    in_=S_T_d[:],
                    in_offset=bass.IndirectOffsetOnAxis(ap=off2[:, 0:1], axis=0),
                )
                # close i,f,g's psum groups first; gate o's k-matmuls then
                # reuse the zpool slot freed by gate i's activation
                for g in range(3):
                    z_outer(g)
                zp[3] = z_kmm(3, zpool, "zbank")
                z_outer(3)

        # ---- batched log_softmax over all 16 steps ----
        mx = state.tile([MAX_LEN, 1], f32)
        nc.vector.reduce_max(mx[:], shist[:], axis=AX.X)
        negm = state.tile([MAX_LEN, 1], f32)
        nc.vector.tensor_scalar_mul(negm[:], mx[:], -1.0)
        etile = work.tile([MAX_LEN, N_ANTES], f32, tag="etile")
        sume = state.tile([MAX_LEN, 1], f32)
        nc.scalar.activation(
            etile[:], shist[:], AF.Exp, bias=negm[:], accum_out=sume[:]
        )
        lsum = state.tile([MAX_LEN, 1], f32)
        nc.scalar.activation(lsum[:], sume[:], AF.Ln)
        offc = state.tile([MAX_LEN, 1], f32)
        nc.vector.tensor_add(offc[:], mx[:], lsum[:])
        nc.vector.tensor_scalar_mul(offc[:], offc[:], -1.0)
        logp = work.tile([MAX_LEN, N_ANTES], f32, tag="logp")
        nc.scalar.activation(logp[:], shist[:], AF.Identity, bias=offc[:])
        nc.sync.dma_start(out=out_d[:], in_=logp[:])

    nc.compile()
    return nc


def pack2(a):
    """[nk*128, X] -> [128, nk, X] so that a[k*128+p, x] == out[p, k, x]."""
    rows, X = a.shape
    nk = rows // 128
    return np.ascontiguousarray(a.reshape(nk, 128, X).transpose(1, 0, 2))


def colpack(v):
    """[nk*128] -> [128, nk] per-partition bias layout."""
    n = v.shape[0]
    nk = n // 128
    return np.ascontiguousarray(v.reshape(nk, 128).T)


def make_in_maps(inputs):
    f32 = np.float32
    f16 = np.float16
    context = np.ascontiguousarray(np.asarray(inputs["context"], f32))
    S = np.ascontiguousarray(np.asarray(inputs["S"], f32))
    enc_w1 = np.asarray(inputs["enc_w1"], f32)
    enc_b1 = np.asarray(inputs["enc_b1"], f32)
    enc_w2 = np.asarray(inputs["enc_w2"], f32)
    enc_b2 = np.asarray(inputs["enc_b2"], f32)
    w_ih = np.asarray(inputs["w_ih"], f32)
    w_hh = np.asarray(inputs["w_hh"], f32)
    b_ih = np.asarray(inputs["b_ih"], f32)
    b_hh = np.asarray(inputs["b_hh"], f32)
    att_w1 = np.asarray(inputs["att_w1"], f32)
    att_b1 = np.asarray(inputs["att_b1"], f32)
    att_w2 = np.asarray(inputs["att_w2"], f32)

    ws = w_ih.sum(axis=1, dtype=np.float64).astype(f32)
    wsb = np.stack([ws, b_ih + b_hh], axis=0)  # [2, 1024]

    shared = {
        "enc_w1T": np.ascontiguousarray(enc_w1.T).astype(f16),
        "enc_b1p": colpack(enc_b1),
        "enc_w2T": pack2(np.ascontiguousarray(enc_w2.T)).astype(f16),
        "enc_b2p": colpack(enc_b2),
        "w_hhT": pack2(np.ascontiguousarray(w_hh.T)).astype(f16),
        "wsb": wsb.astype(f16),
        "WbT": pack2(np.ascontiguousarray(att_w1[:, N_TRAIN:].T) / N_TRAIN).astype(
            f16
        ),
        "ab1p": colpack(att_b1),
        "w2p": colpack(att_w2[0]).astype(f16),
        "ident8": np.eye(8, dtype=f16),
        "off_init": np.array([[0], [N_ANTES]], dtype=np.uint32),
        "iota4": np.arange(4, dtype=f32)[None, :],
    }
    in_maps = []
    for c in range(NCORES):
        rows = slice(c * R, (c + 1) * R)
        m = dict(shared)
        m["ctxT"] = np.ascontiguousarray(context[rows].T).astype(f16)
        m["S_kxn"] = pack2(S[rows]).astype(f16)
        st = np.ascontiguousarray(S[rows].T).astype(f16)
        m["S_T"] = np.concatenate([st, np.ones((1, R), f16)], axis=0)
        m["WaT"] = pack2(np.ascontiguousarray(att_w1[:, :N_TRAIN][:, rows].T)).astype(
            f16
        )
        in_maps.append(m)
    return in_maps


_NC = None


def kernel(**inputs):
    global _NC
    from concourse.bass_utils import run_bass_kernel_spmd

    if _NC is None:
        _NC = build_nc()
    in_maps = make_in_maps(inputs)
    res = run_bass_kernel_spmd(
        _NC, in_maps, list(range(NCORES)),
        trace=bool(int(os.environ.get("KERNEL_TRACE", "0"))),
    )
    out = np.asarray(res.results[0]["out"], np.float32)
    if res.exec_time_ns is not None:
        print(f"HW exec time: {res.exec_time_ns} ns")
    return out


# revision 13
# speedup vs baseline: 1.1302x; 1.0445x over previous
"""Bass/Tile SPMD kernel for nn_CEN_BRL_22763326668900 on 8 trn2 NeuronCores.

Reference computation (see problem):
  phi = relu(ctx @ w1.T + b1) @ w2.T + b2            [4096, 256]
  pre = S.T @ Wa.T                                   [2048, 256]
  16 LSTM steps with x_t rank-1:
      x_0 = ones, x_{t+1} = S[:, idx_t] broadcast over columns
  =>  x @ w_ih.T == outer(s_col, ws), ws = w_ih.sum(axis=1)   (host-precomputed)
  Per step: z = h @ w_hh.T + outer + bias; gates -> c,h; e = mean_rows(h);
  hid = relu(pre + e @ Wb.T + b1a); scores = hid @ w2a.T; logp = log_softmax;
  idx = argmax.

Sharding: data-parallel over 4096 train rows (512/core). Per step one tiny
AllGather of per-core av partials (av = rowsum(h) @ Wb.T/4096). Attention MLP,
softmax and argmax replicated on every core; each core gathers its own
S[:, idx] shard column via indirect DMA from (S_shard.T with a ones row
appended; the second gather offset pins row 1 of the rhs to ones so a single
K=2 matmul applies both the outer product and the gate bias).

Matmuls run in fp16 (full PE rate; 4-byte matmuls hit a walrus codegen bug
with sync waits). Top-2 score margin is ~0.09 vs ~1e-3 worst-case fp16 error,
so the argmax chain is stable; accumulation stays fp32 in PSUM.

Pipelining: the next step's w_hh matmuls for gates i,f,g are emitted right
after the AllGather is issued, so the PE fills the collective latency window;
gate o's matmuls take the attention psum slot after the score matmuls drain.
"""

import os
import sys
import numpy as np

sys.path.insert(0, "/opt/trn_rl_repo")

NCORES = 8
N_TRAIN, N_FEAT, N_HID, ENC = 4096, 64, 256, 256
N_ANTES, ATT_H, MAX_LEN = 2048, 256, 16
R = N_TRAIN // NCORES  # 512 rows per core
G4 = 4 * ENC  # 1024


def build_nc():
    import concourse.bass as bass
    import concourse.bacc as bacc
    import concourse.tile as tile
    from concourse import mybir
    from contextlib import ExitStack

    f32 = mybir.dt.float32
    f16 = mybir.dt.float16
    u32 = mybir.dt.uint32
    AF = mybir.ActivationFunctionType
    ALU = mybir.AluOpType
    AX = mybir.AxisListType

    nc = bacc.Bacc(num_devices=NCORES)

    # ---- kernel I/O ----
    ctxT_d = nc.dram_tensor("ctxT", [N_FEAT, R], f16, kind="ExternalInput")
    S_kxn_d = nc.dram_tensor("S_kxn", [128, 4, N_ANTES], f16, kind="ExternalInput")
    # S_shard.T plus a trailing all-ones row (gather offset 2048 -> ones)
    S_T_d = nc.dram_tensor("S_T", [N_ANTES + 1, R], f16, kind="ExternalInput")
    WaT_d = nc.dram_tensor("WaT", [128, 4, ATT_H], f16, kind="ExternalInput")
    ew1T_d = nc.dram_tensor("enc_w1T", [N_FEAT, N_HID], f16, kind="ExternalInput")
    eb1_d = nc.dram_tensor("enc_b1p", [128, 2], f32, kind="ExternalInput")
    ew2T_d = nc.dram_tensor("enc_w2T", [128, 2, ENC], f16, kind="ExternalInput")
    eb2_d = nc.dram_tensor("enc_b2p", [128, 2], f32, kind="ExternalInput")
    whhT_d = nc.dram_tensor("w_hhT", [128, 2, G4], f16, kind="ExternalInput")
    # row 0 = w_ih.sum(1), row 1 = b_ih + b_hh  (K=2 outer+bias matmul lhsT)
    wsb_d = nc.dram_tensor("wsb", [2, G4], f16, kind="ExternalInput")
    WbT_d = nc.dram_tensor("WbT", [128, 2, ATT_H], f16, kind="ExternalInput")
    ab1_d = nc.dram_tensor("ab1p", [128, 2], f32, kind="ExternalInput")
    w2p_d = nc.dram_tensor("w2p", [128, 2], f16, kind="ExternalInput")
    id8_d = nc.dram_tensor("ident8", [8, 8], f16, kind="ExternalInput")
    # gather offsets init: [0, 2048] (row 1 always reads the ones row)
    offi_d = nc.dram_tensor("off_init", [2, 1], u32, kind="ExternalInput")
    iota4_d = nc.dram_tensor("iota4", [1, 4], f32, kind="ExternalInput")
    out_d = nc.dram_tensor("out", [MAX_LEN, N_ANTES], f32, kind="ExternalOutput")

    # internal DRAM for collectives
    cc_pre_in = nc.dram_tensor("cc_pre_in", [ATT_H, N_ANTES], f16)
    cc_pre_out = nc.dram_tensor(
        "cc_pre_out", [ATT_H, N_ANTES], f16, addr_space="Shared"
    )
    groups = [list(range(NCORES))]

    with tile.TileContext(nc, num_cores=NCORES) as tc, ExitStack() as ctx:
        const = ctx.enter_context(tc.tile_pool(name="const", bufs=1))
        state = ctx.enter_context(tc.tile_pool(name="state", bufs=1))
        work = ctx.enter_context(tc.tile_pool(name="work", bufs=2))
        # 3 two-bank slots for LSTM gate psums (i,f,g prefill; o borrows apool)
        zpool = ctx.enter_context(tc.tile_pool(name="zpool", bufs=3, space="PSUM"))
        apool = ctx.enter_context(tc.tile_pool(name="apool", bufs=2, space="PSUM"))

        # ---- load constants ----
        def load(shape, dram, tag, dt=f32):
            t = const.tile(shape, dt, tag=tag)
            nc.sync.dma_start(out=t[:], in_=dram[:])
            return t

        ctxT = load([N_FEAT, R], ctxT_d, "ctxT", f16)
        ew1T = load([N_FEAT, N_HID], ew1T_d, "ew1T", f16)
        eb1 = load([128, 2], eb1_d, "eb1")
        ew2T = load([128, 2, ENC], ew2T_d, "ew2T", f16)
        eb2 = load([128, 2], eb2_d, "eb2")
        whhT = load([128, 2, G4], whhT_d, "whhT", f16)
        wsb = load([2, G4], wsb_d, "wsb", f16)
        WbT = load([128, 2, ATT_H], WbT_d, "WbT", f16)
        ab1 = load([128, 2], ab1_d, "ab1")
        w2p = load([128, 2], w2p_d, "w2p", f16)
        id8 = load([8, 8], id8_d, "id8", f16)
        iota4 = load([1, 4], iota4_d, "iota4")

        # ---- persistent state ----
        hT = state.tile([128, 2, R], f16)  # h transposed: [enc(128x2), rows]
        cT = state.tile([128, 2, R], f32)
        s_col2 = state.tile([2, R], f16)  # row0 = S_shard[:, idx], row1 = ones
        pre0 = state.tile([128, N_ANTES], f16)  # preT rows 0:128
        pre1 = state.tile([128, N_ANTES], f16)  # preT rows 128:256
        shist = state.tile([MAX_LEN, N_ANTES], f32)  # raw scores per step
        hsum = state.tile([128, 2], f32)
        hsum16 = state.tile([128, 2], f16)
        avcol = state.tile([128, 2], f16)  # send payload (av partial, column)
        mail = state.tile([128, 2, NCORES, 2], f16)  # [p, parity, sender, e]
        avsum = state.tile([128, 2], f32)
        avb2 = state.tile([128, 2], f32)
        off2 = state.tile([2, 1], u32)
        cmax = state.tile([1, 4, 8], f32)  # per-512-chunk top-8 score values
        cmif = state.tile([1, 4], f32)  # per-chunk argmax as f32
        gmx = state.tile([1, 1], f32)
        eqm = state.tile([1, 4], f32)
        cix = state.tile([1, 1], f32)
        msk = state.tile([1, 4], f32)
        jsel = state.tile([1, 1], f32)
        idxf = state.tile([1, 1], f32)

        nc.vector.memset(cT[:], 0.0)
        nc.vector.memset(s_col2[:], 1.0)  # x_0 = ones (and row1 stays ones)
        nc.sync.dma_start(out=off2[:], in_=offi_d[:])

        # remote-dma av exchange: sems + this core's rank for its mail slot
        rsem = nc.alloc_semaphore("av_rsem")  # +2 per arriving sender frame
        lsem = nc.alloc_semaphore("av_lsem")  # +16 per completed local send
        psem = nc.alloc_semaphore("av_psem")  # desc-gen completions
        rank_v = nc.gpsimd.partition_id()

        # ---- phi = encoder(context) -> h_0 (transposed layout) ----
        a1_tiles = []
        for m in range(2):
            p = apool.tile([128, R], f32, tag="abank")
            nc.tensor.matmul(
                p[:], ew1T[:, m * 128 : (m + 1) * 128], ctxT[:],
                start=True, stop=True,
            )
            a1 = work.tile([128, R], f16, tag="a1")
            nc.scalar.activation(a1[:], p[:], AF.Relu, bias=eb1[:, m : m + 1])
            a1_tiles.append(a1)
        for m in range(2):
            p = apool.tile([128, R], f32, tag="abank")
            for k in range(2):
                nc.tensor.matmul(
                    p[:], ew2T[:, k, m * 128 : (m + 1) * 128], a1_tiles[k][:],
                    start=(k == 0), stop=(k == 1),
                )
            nc.scalar.activation(
                hT[:, m, :], p[:], AF.Identity, bias=eb2[:, m : m + 1]
            )

        # ---- pre partial = Wa_shard @ S_shard, then AllReduce (fp16) ----
        with tc.tile_pool(name="spool", bufs=1) as spool:
            S_sb = spool.tile([128, 4, N_ANTES], f16)
            nc.sync.dma_start(out=S_sb[:], in_=S_kxn_d[:])
            WaT_sb = spool.tile([128, 4, ATT_H], f16)
            nc.sync.dma_start(out=WaT_sb[:], in_=WaT_d[:])
            for m in range(2):
                prep = work.tile([128, N_ANTES], f16, tag="hid%d" % m)
                for n in range(4):
                    p = apool.tile([128, 512], f32, tag="abank")
                    for k in range(4):
                        nc.tensor.matmul(
                            p[:],
                            WaT_sb[:, k, m * 128 : (m + 1) * 128],
                            S_sb[:, k, n * 512 : (n + 1) * 512],
                            start=(k == 0), stop=(k == 3),
                        )
                    if n % 2 == 0:
                        nc.scalar.copy(prep[:, n * 512 : (n + 1) * 512], p[:])
                    else:
                        nc.vector.tensor_copy(prep[:, n * 512 : (n + 1) * 512], p[:])
                nc.sync.dma_start(
                    out=cc_pre_in[m * 128 : (m + 1) * 128, :], in_=prep[:]
                )
        nc.gpsimd.collective_compute(
            "AllReduce", ALU.add, replica_groups=groups,
            ins=[cc_pre_in[:]], outs=[cc_pre_out[:]],
        )
        nc.sync.dma_start(out=pre0[:], in_=cc_pre_out[0:128, :])
        nc.sync.dma_start(out=pre1[:], in_=cc_pre_out[128:256, :])

        # ---- 16 LSTM + attention steps (software pipelined on PE) ----
        # zp[g] is gate g's [128, 2, 512] psum: [p, e, i] = z[gate g, 128e+p, i]
        zp = [None] * 4

        def z_kmm(g, pool, tag):
            p = pool.tile([128, 2, 512], f32, tag=tag)
            for e in range(2):
                m = 2 * g + e
                for k in range(2):
                    nc.tensor.matmul(
                        p[:, e, :],
                        whhT[:, k, m * 128 : (m + 1) * 128],
                        hT[:, k, :],
                        start=(k == 0), stop=False,
                    )
            return p

        def z_outer(g):
            for e in range(2):
                m = 2 * g + e
                nc.tensor.matmul(
                    zp[g][:, e, :],
                    wsb[0:2, m * 128 : (m + 1) * 128],
                    s_col2[0:2, :],
                    start=False, stop=True,
                )

        # step 0 prologue: all four gate psums (s_col = ones already valid).
        # Gate o's k-matmuls wait for the zpool slot freed once gate i's
        # activation has consumed its psum, so close i,f,g's groups first.
        for g in range(3):
            zp[g] = z_kmm(g, zpool, "zbank")
        for g in range(3):
            z_outer(g)
        zp[3] = z_kmm(3, zpool, "zbank")
        z_outer(3)

        for t in range(MAX_LEN):
            # ---- gates -> c, h, rowsum(h) ----
            sig = [None] * 4
            for g in range(4):
                func = AF.Tanh if g == 2 else AF.Sigmoid
                gt = work.tile([128, 2, 512], f32, tag="g%d" % g)
                nc.scalar.activation(gt[:], zp[g][:], func)
                sig[g] = gt
            ig = work.tile([128, 2, 512], f32, tag="ig")
            nc.vector.tensor_mul(ig[:], sig[0][:], sig[2][:])
            cf = work.tile([128, 2, 512], f32, tag="cf")
            nc.vector.tensor_mul(cf[:], cT[:], sig[1][:])
            nc.vector.tensor_add(cT[:], ig[:], cf[:])
            th = work.tile([128, 2, 512], f32, tag="th")
            nc.scalar.activation(th[:], cT[:], AF.Tanh)
            nc.vector.tensor_mul(hT[:], sig[3][:], th[:])
            nc.vector.reduce_sum(hsum[:], hT[:], axis=AX.X)
            nc.vector.tensor_copy(hsum16[:], hsum[:])

            # ---- av partial (column layout) + remote broadcast exchange ----
            parity = t % 2
            pav = apool.tile([128, 2], f32, tag="abank")
            for mch in range(2):
                for k in range(2):
                    nc.tensor.matmul(
                        pav[:, mch : mch + 1],
                        WbT[:, k, mch * 128 : (mch + 1) * 128],
                        hsum16[:, k : k + 1],
                        start=(k == 0), stop=(k == 1),
                    )
            if t >= 1:
                # previous send must have drained before reusing the buffer
                nc.vector.wait_ge(lsem, 16 * t)
            nc.vector.tensor_copy(avcol[:], pav[:])
            bc = nc.gpsimd.remote_dma_broadcast(
                mail[:, parity, bass.ds(rank_v, 1), :],
                avcol[:],
                rsem,
                lsem,
                rdests=[(0, k) for k in range(NCORES)],
            )
            bc.then_inc(psem, 1)
            nc.gpsimd.wait_ge(psem, t + 1)
            nc.gpsimd.trigger_dma(count=None)

            # prefill next step's w_hh matmuls for gates i,f,g under the
            # exchange latency
            if t < MAX_LEN - 1:
                for g in range(3):
                    zp[g] = z_kmm(g, zpool, "zbank")

            # all 8 frames landed (2 rsem incs per sender)
            nc.vector.wait_ge(rsem, 16 * (t + 1))
            nc.vector.reduce_sum(
                avsum[:], mail[:, parity, :, :].transpose([0, 2, 1]), axis=AX.X
            )
            nc.vector.tensor_add(avb2[:], avsum[:], ab1[:])

            # ---- hid = relu(pre + avb) ; scores = w2 . hid ----
            hid0 = work.tile([128, N_ANTES], f16, tag="hid0")
            nc.scalar.activation(hid0[:], pre0[:], AF.Relu, bias=avb2[:, 0:1])
            hid1 = work.tile([128, N_ANTES], f16, tag="hid1")
            nc.vector.tensor_scalar(
                hid1[:], pre1[:], avb2[:, 1:2], 0.0, op0=ALU.add, op1=ALU.max
            )
            scs = work.tile([1, N_ANTES], f32, tag="scs")
            last = t == MAX_LEN - 1
            for n in range(4):
                ps = apool.tile([1, 512], f32, tag="abank")
                nc.tensor.matmul(
                    ps[:], w2p[:, 0:1], hid0[:, n * 512 : (n + 1) * 512],
                    start=True, stop=False,
                )
                nc.tensor.matmul(
                    ps[:], w2p[:, 1:2], hid1[:, n * 512 : (n + 1) * 512],
                    start=False, stop=True,
                )
                dst = scs[0:1, n * 512 : (n + 1) * 512]
                nc.scalar.copy(dst, ps[:])
                if not last:
                    # per-chunk top-8 + argmax, pipelined with later chunks
                    nc.vector.max(cmax[:, n, :], dst)
                    mi = work.tile([1, 8], u32, tag="mi%d" % n)
                    nc.vector.max_index(mi[:], cmax[:, n, :], dst)
                    # cmif[n] = 512*n + argmax_n (u32 -> f32 cast + const add)
                    nc.vector.tensor_scalar(
                        cmif[0:1, n : n + 1], mi[0:1, 0:1], float(512 * n),
                        None, op0=ALU.add,
                    )
            nc.sync.dma_start(out=shist[t : t + 1, :], in_=scs[:])

            if not last:
                # combine the 4 chunk argmaxes (single-lane, short rows)
                nc.vector.reduce_max(gmx[:], cmax[0:1, :, 0], axis=AX.X)
                nc.vector.tensor_scalar(
                    eqm[:], cmax[0:1, :, 0], gmx[0:1, 0:1], None,
                    op0=ALU.is_equal,
                )
                nc.vector.tensor_mul(msk[:], eqm[:], cmif[:])
                nc.vector.reduce_max(idxf[:], msk[:], axis=AX.X)
                nc.vector.tensor_copy(off2[0:1, 0:1], idxf[:])
                nc.gpsimd.indirect_dma_start(
                    out=s_col2[:],
                    out_offset=None,
                    in_=S_T_d[:],
                    in_offset=bass.IndirectOffsetOnAxis(ap=off2[:, 0:1], axis=0),
                )
                # close i,f,g's psum groups first; gate o's k-matmuls then
                # reuse the zpool slot freed by gate i's activation
                for g in range(3):
                    z_outer(g)
                zp[3] = z_kmm(3, zpool, "zbank")
                z_outer(3)

        # ---- batched log_softmax over all 16 steps ----
        mx = state.tile([MAX_LEN, 1], f32)
        nc.vector.reduce_max(mx[:], shist[:], axis=AX.X)
        negm = state.tile([MAX_LEN, 1], f32)
        nc.vector.tensor_scalar_mul(negm[:], mx[:], -1.0)
        etile = work.tile([MAX_LEN, N_ANTES], f32, tag="etile")
        sume = state.tile([MAX_LEN, 1], f32)
        nc.scalar.activation(
            etile[:], shist[:], AF.Exp, bias=negm[:], accum_out=sume[:]
        )
        lsum = state.tile([MAX_LEN, 1], f32)
        nc.scalar.activation(lsum[:], sume[:], AF.Ln)
        offc = state.tile([MAX_LEN, 1], f32)
        nc.vector.tensor_add(offc[:], mx[:], lsum[:])
        nc.vector.tensor_scalar_mul(offc[:], offc[:], -1.0)
        logp = work.tile([MAX_LEN, N_ANTES], f32, tag="logp")
        nc.scalar.activation(logp[:], shist[:], AF.Identity, bias=offc[:])
        nc.sync.dma_start(out=out_d[:], in_=logp[:])

    nc.compile()
    return nc


def pack2(a):
    """[nk*128, X] -> [128, nk, X] so that a[k*128+p, x] == out[p, k, x]."""
    rows, X = a.shape
    nk = rows // 128
    return np.ascontiguousarray(a.reshape(nk, 128, X).transpose(1, 0, 2))


def colpack(v):
    """[nk*128] -> [128, nk] per-partition bias layout."""
    n = v.shape[0]
    nk = n // 128
    return np.ascontiguousarray(v.reshape(nk, 128).T)


def make_in_maps(inputs):
    f32 = np.float32
    f16 = np.float16
    context = np.ascontiguousarray(np.asarray(inputs["context"], f32))
    S = np.ascontiguousarray(np.asarray(inputs["S"], f32))
    enc_w1 = np.asarray(inputs["enc_w1"], f32)
    enc_b1 = np.asarray(inputs["enc_b1"], f32)
    enc_w2 = np.asarray(inputs["enc_w2"], f32)
    enc_b2 = np.asarray(inputs["enc_b2"], f32)
    w_ih = np.asarray(inputs["w_ih"], f32)
    w_hh = np.asarray(inputs["w_hh"], f32)
    b_ih = np.asarray(inputs["b_ih"], f32)
    b_hh = np.asarray(inputs["b_hh"], f32)
    att_w1 = np.asarray(inputs["att_w1"], f32)
    att_b1 = np.asarray(inputs["att_b1"], f32)
    att_w2 = np.asarray(inputs["att_w2"], f32)

    ws = w_ih.sum(axis=1, dtype=np.float64).astype(f32)
    wsb = np.stack([ws, b_ih + b_hh], axis=0)  # [2, 1024]

    shared = {
        "enc_w1T": np.ascontiguousarray(enc_w1.T).astype(f16),
        "enc_b1p": colpack(enc_b1),
        "enc_w2T": pack2(np.ascontiguousarray(enc_w2.T)).astype(f16),
        "enc_b2p": colpack(enc_b2),
        "w_hhT": pack2(np.ascontiguousarray(w_hh.T)).astype(f16),
        "wsb": wsb.astype(f16),
        "WbT": pack2(np.ascontiguousarray(att_w1[:, N_TRAIN:].T) / N_TRAIN).astype(
            f16
        ),
        "ab1p": colpack(att_b1),
        "w2p": colpack(att_w2[0]).astype(f16),
        "ident8": np.eye(8, dtype=f16),
        "off_init": np.array([[0], [N_ANTES]], dtype=np.uint32),
        "iota4": np.arange(4, dtype=f32)[None, :],
    }
    in_maps = []
    for c in range(NCORES):
        rows = slice(c * R, (c + 1) * R)
        m = dict(shared)
        m["ctxT"] = np.ascontiguousarray(context[rows].T).astype(f16)
        m["S_kxn"] = pack2(S[rows]).astype(f16)
        st = np.ascontiguousarray(S[rows].T).astype(f16)
        m["S_T"] = np.concatenate([st, np.ones((1, R), f16)], axis=0)
        m["WaT"] = pack2(np.ascontiguousarray(att_w1[:, :N_TRAIN][:, rows].T)).astype(
            f16
        )
        in_maps.append(m)
    return in_maps


_NC = None


def kernel(**inputs):
    global _NC
    from concourse.bass_utils import run_bass_kernel_spmd

    if _NC is None:
        _NC = build_nc()
    in_maps = make_in_maps(inputs)
    res = run_bass_kernel_spmd(
        _NC, in_maps, list(range(NCORES)),
        trace=bool(int(os.environ.get("KERNEL_TRACE", "0"))),
    )
    out = np.asarray(res.results[0]["out"], np.float32)
    if res.exec_time_ns is not None:
        print(f"HW exec time: {res.exec_time_ns} ns")
    return out


# revision 15
# speedup vs baseline: 1.1439x; 1.0121x over previous
"""Bass/Tile SPMD kernel for nn_CEN_BRL_22763326668900 on 8 trn2 NeuronCores.

Reference computation (see problem):
  phi = relu(ctx @ w1.T + b1) @ w2.T + b2            [4096, 256]
  pre = S.T @ Wa.T                                   [2048, 256]
  16 LSTM steps with x_t rank-1:
      x_0 = ones, x_{t+1} = S[:, idx_t] broadcast over columns
  =>  x @ w_ih.T == outer(s_col, ws), ws = w_ih.sum(axis=1)   (host-precomputed)
  Per step: z = h @ w_hh.T + outer + bias; gates -> c,h; e = mean_rows(h);
  hid = relu(pre + e @ Wb.T + b1a); scores = hid @ w2a.T; logp = log_softmax;
  idx = argmax.

Sharding: data-parallel over 4096 train rows (512/core). Per step one tiny
AllGather of per-core av partials (av = rowsum(h) @ Wb.T/4096). Attention MLP,
softmax and argmax replicated on every core; each core gathers its own
S[:, idx] shard column via indirect DMA from (S_shard.T with a ones row
appended; the second gather offset pins row 1 of the rhs to ones so a single
K=2 matmul applies both the outer product and the gate bias).

Matmuls run in fp16 (full PE rate; 4-byte matmuls hit a walrus codegen bug
with sync waits). Top-2 score margin is ~0.09 vs ~1e-3 worst-case fp16 error,
so the argmax chain is stable; accumulation stays fp32 in PSUM.

Pipelining: the next step's w_hh matmuls for gates i,f,g are emitted right
after the AllGather is issued, so the PE fills the collective latency window;
gate o's matmuls take the attention psum slot after the score matmuls drain.
"""

import os
import sys
import numpy as np

sys.path.insert(0, "/opt/trn_rl_repo")

NCORES = 8
N_TRAIN, N_FEAT, N_HID, ENC = 4096, 64, 256, 256
N_ANTES, ATT_H, MAX_LEN = 2048, 256, 16
R = N_TRAIN // NCORES  # 512 rows per core
G4 = 4 * ENC  # 1024


def build_nc():
    import concourse.bass as bass
    import concourse.bacc as bacc
    import concourse.tile as tile
    from concourse import mybir
    from contextlib import ExitStack

    f32 = mybir.dt.float32
    f16 = mybir.dt.float16
    u32 = mybir.dt.uint32
    AF = mybir.ActivationFunctionType
    ALU = mybir.AluOpType
    AX = mybir.AxisListType

    nc = bacc.Bacc(num_devices=NCORES)

    # ---- kernel I/O ----
    ctxT_d = nc.dram_tensor("ctxT", [N_FEAT, R], f16, kind="ExternalInput")
    S_kxn_d = nc.dram_tensor("S_kxn", [128, 4, N_ANTES], f16, kind="ExternalInput")
    # S_shard.T plus a trailing all-ones row (gather offset 2048 -> ones)
    S_T_d = nc.dram_tensor("S_T", [N_ANTES + 1, R], f16, kind="ExternalInput")
    WaT_d = nc.dram_tensor("WaT", [128, 4, ATT_H], f16, kind="ExternalInput")
    ew1T_d = nc.dram_tensor("enc_w1T", [N_FEAT, N_HID], f16, kind="ExternalInput")
    eb1_d = nc.dram_tensor("enc_b1p", [128, 2], f32, kind="ExternalInput")
    ew2T_d = nc.dram_tensor("enc_w2T", [128, 2, ENC], f16, kind="ExternalInput")
    eb2_d = nc.dram_tensor("enc_b2p", [128, 2], f32, kind="ExternalInput")
    whhT_d = nc.dram_tensor("w_hhT", [128, 2, G4], f16, kind="ExternalInput")
    # row 0 = w_ih.sum(1), row 1 = b_ih + b_hh  (K=2 outer+bias matmul lhsT)
    wsb_d = nc.dram_tensor("wsb", [2, G4], f16, kind="ExternalInput")
    WbT_d = nc.dram_tensor("WbT", [128, 2, ATT_H], f16, kind="ExternalInput")
    ab1_d = nc.dram_tensor("ab1p", [128, 2], f32, kind="ExternalInput")
    w2p_d = nc.dram_tensor("w2p", [128, 2], f16, kind="ExternalInput")
    id8_d = nc.dram_tensor("ident8", [8, 8], f16, kind="ExternalInput")
    # gather offsets init: [0, 2048] (row 1 always reads the ones row)
    offi_d = nc.dram_tensor("off_init", [2, 1], u32, kind="ExternalInput")
    iota4_d = nc.dram_tensor("iota4", [1, 4], f32, kind="ExternalInput")
    out_d = nc.dram_tensor("out", [MAX_LEN, N_ANTES], f32, kind="ExternalOutput")

    # internal DRAM for collectives
    cc_av_in = nc.dram_tensor("cc_av_in", [1, ATT_H], f16)
    cc_av_out = nc.dram_tensor("cc_av_out", [NCORES, ATT_H], f16, addr_space="Shared")
    cc_pre_in = nc.dram_tensor("cc_pre_in", [ATT_H, N_ANTES], f16)
    cc_pre_out = nc.dram_tensor(
        "cc_pre_out", [ATT_H, N_ANTES], f16, addr_space="Shared"
    )
    groups = [list(range(NCORES))]

    with tile.TileContext(nc, num_cores=NCORES) as tc, ExitStack() as ctx:
        const = ctx.enter_context(tc.tile_pool(name="const", bufs=1))
        state = ctx.enter_context(tc.tile_pool(name="state", bufs=1))
        work = ctx.enter_context(tc.tile_pool(name="work", bufs=2))
        # 3 two-bank slots for LSTM gate psums (i,f,g prefill; o borrows apool)
        zpool = ctx.enter_context(tc.tile_pool(name="zpool", bufs=3, space="PSUM"))
        apool = ctx.enter_context(tc.tile_pool(name="apool", bufs=2, space="PSUM"))

        # ---- load constants ----
        def load(shape, dram, tag, dt=f32):
            t = const.tile(shape, dt, tag=tag)
            nc.sync.dma_start(out=t[:], in_=dram[:])
            return t

        ctxT = load([N_FEAT, R], ctxT_d, "ctxT", f16)
        ew1T = load([N_FEAT, N_HID], ew1T_d, "ew1T", f16)
        eb1 = load([128, 2], eb1_d, "eb1")
        ew2T = load([128, 2, ENC], ew2T_d, "ew2T", f16)
        eb2 = load([128, 2], eb2_d, "eb2")
        whhT = load([128, 2, G4], whhT_d, "whhT", f16)
        wsb = load([2, G4], wsb_d, "wsb", f16)
        WbT = load([128, 2, ATT_H], WbT_d, "WbT", f16)
        ab1 = load([128, 2], ab1_d, "ab1")
        w2p = load([128, 2], w2p_d, "w2p", f16)
        id8 = load([8, 8], id8_d, "id8", f16)
        iota4 = load([1, 4], iota4_d, "iota4")

        # ---- persistent state ----
        hT = state.tile([128, 2, R], f16)  # h transposed: [enc(128x2), rows]
        cT = state.tile([128, 2, R], f32)
        s_col2 = state.tile([2, R], f16)  # row0 = S_shard[:, idx], row1 = ones
        pre0 = state.tile([128, N_ANTES], f16)  # preT rows 0:128
        pre1 = state.tile([128, N_ANTES], f16)  # preT rows 128:256
        shist = state.tile([MAX_LEN, N_ANTES], f16)  # raw scores per step
        hsum = state.tile([128, 2], f32)
        hsum16 = state.tile([128, 2], f16)
        avb = state.tile([128, 2, 1], f32)
        off2 = state.tile([2, 1], u32)
        cmax = state.tile([1, 4, 8], f16)  # per-512-chunk top-8 score values
        cmif = state.tile([1, 4], f32)  # per-chunk argmax as f32
        gmx = state.tile([1, 1], f32)
        eqm = state.tile([1, 4], f32)
        cix = state.tile([1, 1], f32)
        msk = state.tile([1, 4], f32)
        jsel = state.tile([1, 1], f32)
        idxf = state.tile([1, 1], f32)

        nc.vector.memset(cT[:], 0.0)
        nc.vector.memset(s_col2[:], 1.0)  # x_0 = ones (and row1 stays ones)
        nc.sync.dma_start(out=off2[:], in_=offi_d[:])

        # ---- phi = encoder(context) -> h_0 (transposed layout) ----
        a1_tiles = []
        for m in range(2):
            p = apool.tile([128, R], f32, tag="abank")
            nc.tensor.matmul(
                p[:], ew1T[:, m * 128 : (m + 1) * 128], ctxT[:],
                start=True, stop=True,
            )
            a1 = work.tile([128, R], f16, tag="a1")
            nc.scalar.activation(a1[:], p[:], AF.Relu, bias=eb1[:, m : m + 1])
            a1_tiles.append(a1)
        for m in range(2):
            p = apool.tile([128, R], f32, tag="abank")
            for k in range(2):
                nc.tensor.matmul(
                    p[:], ew2T[:, k, m * 128 : (m + 1) * 128], a1_tiles[k][:],
                    start=(k == 0), stop=(k == 1),
                )
            nc.scalar.activation(
                hT[:, m, :], p[:], AF.Identity, bias=eb2[:, m : m + 1]
            )

        # ---- pre partial = Wa_shard @ S_shard, then AllReduce (fp16) ----
        with tc.tile_pool(name="spool", bufs=1) as spool:
            S_sb = spool.tile([128, 4, N_ANTES], f16)
            nc.sync.dma_start(out=S_sb[:], in_=S_kxn_d[:])
            WaT_sb = spool.tile([128, 4, ATT_H], f16)
            nc.sync.dma_start(out=WaT_sb[:], in_=WaT_d[:])
            for m in range(2):
                prep = work.tile([128, N_ANTES], f16, tag="hid%d" % m)
                for n in range(4):
                    p = apool.tile([128, 512], f32, tag="abank")
                    for k in range(4):
                        nc.tensor.matmul(
                            p[:],
                            WaT_sb[:, k, m * 128 : (m + 1) * 128],
                            S_sb[:, k, n * 512 : (n + 1) * 512],
                            start=(k == 0), stop=(k == 3),
                        )
                    if n % 2 == 0:
                        nc.scalar.copy(prep[:, n * 512 : (n + 1) * 512], p[:])
                    else:
                        nc.vector.tensor_copy(prep[:, n * 512 : (n + 1) * 512], p[:])
                nc.sync.dma_start(
                    out=cc_pre_in[m * 128 : (m + 1) * 128, :], in_=prep[:]
                )
        nc.gpsimd.collective_compute(
            "AllReduce", ALU.add, replica_groups=groups,
            ins=[cc_pre_in[:]], outs=[cc_pre_out[:]],
        )
        nc.sync.dma_start(out=pre0[:], in_=cc_pre_out[0:128, :])
        nc.sync.dma_start(out=pre1[:], in_=cc_pre_out[128:256, :])

        # ---- 16 LSTM + attention steps (software pipelined on PE) ----
        # zp[g] is gate g's [128, 2, 512] psum: [p, e, i] = z[gate g, 128e+p, i]
        zp = [None] * 4

        def z_kmm(g, pool, tag):
            p = pool.tile([128, 2, 512], f32, tag=tag)
            for e in range(2):
                m = 2 * g + e
                for k in range(2):
                    nc.tensor.matmul(
                        p[:, e, :],
                        whhT[:, k, m * 128 : (m + 1) * 128],
                        hT[:, k, :],
                        start=(k == 0), stop=False,
                    )
            return p

        def z_outer(g):
            for e in range(2):
                m = 2 * g + e
                nc.tensor.matmul(
                    zp[g][:, e, :],
                    wsb[0:2, m * 128 : (m + 1) * 128],
                    s_col2[0:2, :],
                    start=False, stop=True,
                )

        # step 0 prologue: all four gate psums (s_col = ones already valid).
        # Gate o's k-matmuls wait for the zpool slot freed once gate i's
        # activation has consumed its psum, so close i,f,g's groups first.
        for g in range(3):
            zp[g] = z_kmm(g, zpool, "zbank")
        for g in range(3):
            z_outer(g)
        zp[3] = z_kmm(3, zpool, "zbank")
        z_outer(3)

        for t in range(MAX_LEN):
            # ---- gates -> c, h, rowsum(h) ----
            sig = [None] * 4
            for g in range(4):
                func = AF.Tanh if g == 2 else AF.Sigmoid
                gt = work.tile([128, 2, 512], f32, tag="g%d" % g)
                nc.scalar.activation(gt[:], zp[g][:], func)
                sig[g] = gt
            ig = work.tile([128, 2, 512], f32, tag="ig")
            nc.vector.tensor_mul(ig[:], sig[0][:], sig[2][:])
            cf = work.tile([128, 2, 512], f32, tag="cf")
            nc.vector.tensor_mul(cf[:], cT[:], sig[1][:])
            nc.vector.tensor_add(cT[:], ig[:], cf[:])
            th = work.tile([128, 2, 512], f32, tag="th")
            nc.scalar.activation(th[:], cT[:], AF.Tanh)
            nc.vector.tensor_mul(hT[:], sig[3][:], th[:])
            nc.vector.reduce_sum(hsum[:], hT[:], axis=AX.X)
            nc.vector.tensor_copy(hsum16[:], hsum[:])

            # ---- av partial + AllGather ----
            pav = apool.tile([1, ATT_H], f32, tag="abank")
            nc.tensor.matmul(
                pav[:], hsum16[:, 0:1], WbT[:, 0, :], start=True, stop=False
            )
            nc.tensor.matmul(
                pav[:], hsum16[:, 1:2], WbT[:, 1, :], start=False, stop=True
            )
            avp_sb = work.tile([1, ATT_H], f16, tag="avp")
            nc.vector.tensor_copy(avp_sb[:], pav[:])
            nc.sync.dma_start(out=cc_av_in[:], in_=avp_sb[:])
            nc.gpsimd.collective_compute(
                "AllGather", ALU.bypass, replica_groups=groups,
                ins=[cc_av_in[:]], outs=[cc_av_out[:]],
            )

            # prefill next step's w_hh matmuls for gates i,f,g under the AG
            if t < MAX_LEN - 1:
                for g in range(3):
                    zp[g] = z_kmm(g, zpool, "zbank")

            av_all = work.tile([NCORES, ATT_H], f16, tag="avall")
            nc.sync.dma_start(out=av_all[:], in_=cc_av_out[:])
            for e in range(2):
                pt = apool.tile([128, 8], f16, tag="abank")
                nc.tensor.transpose(
                    pt[:], av_all[0:NCORES, e * 128 : (e + 1) * 128], id8[:]
                )
                avs = work.tile([128, 1], f32, tag="avs%d" % e)
                nc.vector.reduce_sum(avs[:], pt[:], axis=AX.X)
                nc.vector.tensor_add(avb[:, e, :], avs[:], ab1[:, e : e + 1])

            # ---- hid = relu(pre + avb) ; scores = w2 . hid ----
            hid0 = work.tile([128, N_ANTES], f16, tag="hid0")
            nc.scalar.activation(
                hid0[:, 0:1024], pre0[:, 0:1024], AF.Relu, bias=avb[:, 0, :]
            )
            nc.vector.tensor_scalar(
                hid0[:, 1024:2048], pre0[:, 1024:2048], avb[:, 0, :], 0.0,
                op0=ALU.add, op1=ALU.max,
            )
            hid1 = work.tile([128, N_ANTES], f16, tag="hid1")
            nc.vector.tensor_scalar(
                hid1[:], pre1[:], avb[:, 1, :], 0.0, op0=ALU.add, op1=ALU.max
            )
            scs = work.tile([1, N_ANTES], f16, tag="scs")
            last = t == MAX_LEN - 1
            for n in range(4):
                ps = apool.tile([1, 512], f32, tag="abank")
                nc.tensor.matmul(
                    ps[:], w2p[:, 0:1], hid0[:, n * 512 : (n + 1) * 512],
                    start=True, stop=False,
                )
                nc.tensor.matmul(
                    ps[:], w2p[:, 1:2], hid1[:, n * 512 : (n + 1) * 512],
                    start=False, stop=True,
                )
                dst = scs[0:1, n * 512 : (n + 1) * 512]
                nc.scalar.copy(dst, ps[:])
                if not last:
                    # per-chunk top-8 + argmax, pipelined with later chunks
                    nc.vector.max(cmax[:, n, :], dst)
                    mi = work.tile([1, 8], u32, tag="mi%d" % n)
                    nc.vector.max_index(mi[:], cmax[:, n, :], dst)
                    # cmif[n] = 512*n + argmax_n (u32 -> f32 cast + const add)
                    nc.vector.tensor_scalar(
                        cmif[0:1, n : n + 1], mi[0:1, 0:1], float(512 * n),
                        None, op0=ALU.add,
                    )
            nc.sync.dma_start(out=shist[t : t + 1, :], in_=scs[:])

            if not last:
                # combine the 4 chunk argmaxes (single-lane, short rows)
                nc.vector.reduce_max(gmx[:], cmax[0:1, :, 0], axis=AX.X)
                nc.vector.tensor_scalar(
                    eqm[:], cmax[0:1, :, 0], gmx[0:1, 0:1], None,
                    op0=ALU.is_equal,
                )
                nc.vector.tensor_mul(msk[:], eqm[:], cmif[:])
                nc.vector.reduce_max(idxf[:], msk[:], axis=AX.X)
                nc.vector.tensor_copy(off2[0:1, 0:1], idxf[:])
                nc.gpsimd.indirect_dma_start(
                    out=s_col2[:],
                    out_offset=None,
                    in_=S_T_d[:],
                    in_offset=bass.IndirectOffsetOnAxis(ap=off2[:, 0:1], axis=0),
                )
                # close i,f,g's psum groups first; gate o's k-matmuls then
                # reuse the zpool slot freed by gate i's activation
                for g in range(3):
                    z_outer(g)
                zp[3] = z_kmm(3, zpool, "zbank")
                z_outer(3)

        # ---- batched log_softmax over all 16 steps ----
        mx = state.tile([MAX_LEN, 1], f32)
        nc.vector.reduce_max(mx[:], shist[:], axis=AX.X)
        negm = state.tile([MAX_LEN, 1], f32)
        nc.vector.tensor_scalar_mul(negm[:], mx[:], -1.0)
        etile = work.tile([MAX_LEN, N_ANTES], f32, tag="etile")
        sume = state.tile([MAX_LEN, 1], f32)
        nc.scalar.activation(
            etile[:], shist[:], AF.Exp, bias=negm[:], accum_out=sume[:]
        )
        lsum = state.tile([MAX_LEN, 1], f32)
        nc.scalar.activation(lsum[:], sume[:], AF.Ln)
        offc = state.tile([MAX_LEN, 1], f32)
        nc.vector.tensor_add(offc[:], mx[:], lsum[:])
        nc.vector.tensor_scalar_mul(offc[:], offc[:], -1.0)
        logp = work.tile([MAX_LEN, N_ANTES], f32, tag="logp")
        nc.scalar.activation(logp[:], shist[:], AF.Identity, bias=offc[:])
        nc.sync.dma_start(out=out_d[:], in_=logp[:])

    nc.compile()
    return nc


def pack2(a):
    """[nk*128, X] -> [128, nk, X] so that a[k*128+p, x] == out[p, k, x]."""
    rows, X = a.shape
    nk = rows // 128
    return np.ascontiguousarray(a.reshape(nk, 128, X).transpose(1, 0, 2))


def colpack(v):
    """[nk*128] -> [128, nk] per-partition bias layout."""
    n = v.shape[0]
    nk = n // 128
    return np.ascontiguousarray(v.reshape(nk, 128).T)


def make_in_maps(inputs):
    f32 = np.float32
    f16 = np.float16
    context = np.ascontiguousarray(np.asarray(inputs["context"], f32))
    S = np.ascontiguousarray(np.asarray(inputs["S"], f32))
    enc_w1 = np.asarray(inputs["enc_w1"], f32)
    enc_b1 = np.asarray(inputs["enc_b1"], f32)
    enc_w2 = np.asarray(inputs["enc_w2"], f32)
    enc_b2 = np.asarray(inputs["enc_b2"], f32)
    w_ih = np.asarray(inputs["w_ih"], f32)
    w_hh = np.asarray(inputs["w_hh"], f32)
    b_ih = np.asarray(inputs["b_ih"], f32)
    b_hh = np.asarray(inputs["b_hh"], f32)
    att_w1 = np.asarray(inputs["att_w1"], f32)
    att_b1 = np.asarray(inputs["att_b1"], f32)
    att_w2 = np.asarray(inputs["att_w2"], f32)

    ws = w_ih.sum(axis=1, dtype=np.float64).astype(f32)
    wsb = np.stack([ws, b_ih + b_hh], axis=0)  # [2, 1024]

    shared = {
        "enc_w1T": np.ascontiguousarray(enc_w1.T).astype(f16),
        "enc_b1p": colpack(enc_b1),
        "enc_w2T": pack2(np.ascontiguousarray(enc_w2.T)).astype(f16),
        "enc_b2p": colpack(enc_b2),
        "w_hhT": pack2(np.ascontiguousarray(w_hh.T)).astype(f16),
        "wsb": wsb.astype(f16),
        "WbT": pack2(np.ascontiguousarray(att_w1[:, N_TRAIN:].T) / N_TRAIN).astype(
            f16
        ),
        "ab1p": colpack(att_b1),
        "w2p": colpack(att_w2[0]).astype(f16),
        "ident8": np.eye(8, dtype=f16),
        "off_init": np.array([[0], [N_ANTES]], dtype=np.uint32),
        "iota4": np.arange(4, dtype=f32)[None, :],
    }
    in_maps = []
    for c in range(NCORES):
        rows = slice(c * R, (c + 1) * R)
        m = dict(shared)
        m["ctxT"] = np.ascontiguousarray(context[rows].T).astype(f16)
        m["S_kxn"] = pack2(S[rows]).astype(f16)
        st = np.ascontiguousarray(S[rows].T).astype(f16)
        m["S_T"] = np.concatenate([st, np.ones((1, R), f16)], axis=0)
        m["WaT"] = pack2(np.ascontiguousarray(att_w1[:, :N_TRAIN][:, rows].T)).astype(
            f16
        )
        in_maps.append(m)
    return in_maps


_NC = None


def kernel(**inputs):
    global _NC
    from concourse.bass_utils import run_bass_kernel_spmd

    if _NC is None:
        _NC = build_nc()
    in_maps = make_in_maps(inputs)
    res = run_bass_kernel_spmd(
        _NC, in_maps, list(range(NCORES)),
        trace=bool(int(os.environ.get("KERNEL_TRACE", "0"))),
    )
    out = np.asarray(res.results[0]["out"], np.float32)
    if res.exec_time_ns is not None:
        print(f"HW exec time: {res.exec_time_ns} ns")
    return out
